# revision 1
# baseline (speedup 1.0000x reference)
"""LoRA multi-head attention kernel for 8 Trainium2 NeuronCores.

Problem: q = x_q@(Wq.T + Aq@Bq*2) + bq ; k = x_k@Wk.T + bk ;
         v = x_v@(Wv.T + Av@Bv*2) + bv ; MHA over 16 heads, D=64,
         out = attn_out @ Wo.T + bo.   Shapes: x [2048, 4, 1024].

Sharding: core c handles batch b = c//2 and head-group hg = c%2
(8 heads = 512 channels). LoRA weights are merged on the host
(mathematically exact), the 1/sqrt(D) score scale is folded into Wk/bk,
and x is transposed on the host so every matmul contracts over the
partition dimension. Each core computes a partial output
(its 512 channels through Wo); the host sums the two partials per batch.

Device layout per core:
  qT/kT  [ch, tok] ; v [tok, ch] augmented with a ones column so the
  attn@v matmul also produces the softmax denominator (scores are
  exponentiated WITHOUT max subtraction -- safe here, |scores| < ~6 --
  and normalization happens after attn@v on the [D, S] output, 32x
  cheaper than normalizing the attention matrix).
All matmuls run as float32r (full PE rate at free dim 512).
"""

import sys

import numpy as np

sys.path.insert(0, "/opt/trn_rl_repo")

from contextlib import ExitStack  # noqa: E402

import concourse.bass as bass  # noqa: E402
import concourse.tile as tile  # noqa: E402
from concourse import bacc, mybir  # noqa: E402
from concourse.bass_utils import run_bass_kernel_spmd  # noqa: E402

F32 = mybir.dt.float32
F32R = mybir.dt.float32r
AF = mybir.ActivationFunctionType
ALU = mybir.AluOpType

E = 1024
D = 64
NHC = 8            # heads per core
CH = NHC * D       # 512 output channels per core
KT = E // 128      # k-tiles over the E contraction
NCORES = 8
B = 4


def build_program(S=2048, num_devices=8):
    TB = 256 if S >= 512 else S     # token block for projections
    NTB = S // TB
    NSB = S // 512 if S >= 512 else 1
    SBK = S // NSB                  # s-block width
    NTT = S // 128                  # t tiles
    MT = S // 128                   # tok tiles (v projection / output)
    NM = CH // 128                  # ch tiles per core (4)

    nc = bacc.Bacc(
        "TRN2", target_bir_lowering=False, debug=False, num_devices=num_devices
    )

    def dram(name, shape, out=False, dt=F32):
        kind = "ExternalOutput" if out else "ExternalInput"
        return nc.dram_tensor(name, shape, dt, kind=kind).ap()

    xq = dram("xq", [128, KT, S], dt=F32R)
    xk = dram("xk", [128, KT, S], dt=F32R)
    xv = dram("xv", [128, KT, S], dt=F32R)
    wq = dram("wq", [128, KT, CH], dt=F32R)
    wk = dram("wk", [128, KT, CH], dt=F32R)
    wv = dram("wv", [128, KT, CH], dt=F32R)
    wo = dram("wo", [128, NM, E // 512, 512], dt=F32R)
    bq = dram("bq", [128, NM])
    bk = dram("bk", [128, NM])
    bv = dram("bv", [128, CH])
    bo = dram("bo", [128, E])
    onesd = dram("onesd", [64], dt=F32R)
    out = dram("out", [S, E], out=True)

    with tile.TileContext(nc) as tc, ExitStack() as top:
        persist = top.enter_context(tc.tile_pool(name="persist", bufs=1))
        qT = persist.tile([128, NM, S], F32R)          # [ch%128, ch//128, tok]
        kT = persist.tile([128, NM, S], F32R)
        vaug = persist.tile([128, NTT, NHC, D + 1], F32R)  # [tok%128, ttile, h, d+1]
        aoT = persist.tile([128, NM, S], F32R)         # attention out, [ch, tok]
        bq_sb = persist.tile([128, NM], F32)
        bk_sb = persist.tile([128, NM], F32)
        bv_sb = persist.tile([128, CH], F32)
        ones_sb = persist.tile([1, D], F32R)
        nc.sync.dma_start(out=bq_sb, in_=bq)
        nc.sync.dma_start(out=bk_sb, in_=bk)
        nc.sync.dma_start(out=bv_sb, in_=bv)
        nc.gpsimd.dma_start(out=ones_sb, in_=onesd[None, :])
        nc.vector.memset(vaug[:, :, :, D:D + 1].bitcast(F32), 1.0)

        # ---------------- Phase A: q/k/v projections ----------------
        with tc.tile_pool(name="wts", bufs=1) as wpool, \
             tc.tile_pool(name="xs", bufs=3) as xpool, \
             tc.tile_pool(name="pps", bufs=3, space="PSUM") as ppool:
            wq_sb = wpool.tile([128, KT, CH], F32R, tag="wq")
            wk_sb = wpool.tile([128, KT, CH], F32R, tag="wk")
            wv_sb = wpool.tile([128, KT, CH], F32R, tag="wv")
            nc.sync.dma_start(out=wq_sb, in_=wq)
            nc.sync.dma_start(out=wk_sb, in_=wk)
            nc.sync.dma_start(out=wv_sb, in_=wv)

            # k then q: qT/kT[ch, tok] = W.T @ x.T  (+ bias per partition)
            for xap, w_sb, b_sb, dst in (
                (xk, wk_sb, bk_sb, kT),
                (xq, wq_sb, bq_sb, qT),
            ):
                for nb in range(NTB):
                    xt = xpool.tile([128, KT, TB], F32R, tag="x")
                    nc.sync.dma_start(out=xt, in_=xap[:, :, nb * TB:(nb + 1) * TB])
                    for m in range(NM):
                        ps = ppool.tile([128, TB], F32, tag="pp")
                        for k in range(KT):
                            nc.tensor.matmul(
                                ps,
                                (w_sb[:, k, m * 128:(m + 1) * 128]),
                                (xt[:, k, :]),
                                start=(k == 0),
                                stop=(k == KT - 1),
                            )
                        nc.vector.tensor_scalar(
                            out=dst[:, m, nb * TB:(nb + 1) * TB],
                            in0=ps,
                            scalar1=b_sb[:, m:m + 1],
                            scalar2=None,
                            op0=ALU.add,
                        )
            # v: v[tok, ch] = x @ Wv_eff  (+ bias along free dim)
            for nb in range(NTB):
                xt = xpool.tile([128, KT, TB], F32R, tag="x")
                nc.sync.dma_start(out=xt, in_=xv[:, :, nb * TB:(nb + 1) * TB])
                for mi in range(TB // 128):
                    mt = nb * (TB // 128) + mi
                    ps = ppool.tile([128, CH], F32, tag="pp")
                    for k in range(KT):
                        nc.tensor.matmul(
                            ps,
                            (xt[:, k, mi * 128:(mi + 1) * 128]),
                            (wv_sb[:, k, :]),
                            start=(k == 0),
                            stop=(k == KT - 1),
                        )
                    nc.vector.tensor_add(
                        out=vaug[:, mt, :, 0:D],
                        in0=ps.rearrange("p (h d) -> p h d", d=D),
                        in1=bv_sb.rearrange("p (h d) -> p h d", d=D),
                    )

        # ---------------- Phase B: attention ----------------
        # scores_T[t, s] = k_scaled @ q.T per head; exp; oaug = [v | 1].T @ exp
        # (row D of oaug = softmax denominator); normalize into aoT.
        with tc.tile_pool(name="scps", bufs=1, space="PSUM") as scpool, \
             tc.tile_pool(name="oaps", bufs=1, space="PSUM") as opool, \
             tc.tile_pool(name="bcps", bufs=1, space="PSUM") as bcpool, \
             tc.tile_pool(name="exs", bufs=4) as expool, \
             tc.tile_pool(name="nrm", bufs=3) as npool:
            for hp in range(NM):
                for sb_i in range(NSB):
                    ssl = slice(sb_i * SBK, (sb_i + 1) * SBK)
                    oaugs = [
                        opool.tile(
                            [D + 1, SBK], F32, tag=f"oaug{h_in}", name=f"oaug{h_in}"
                        )
                        for h_in in range(2)
                    ]
                    for tt2 in range(NTT // 2):
                        for h_in in range(2):
                            h = 2 * hp + h_in
                            p0 = h_in * 64
                            sc = scpool.tile([128, 2, SBK], F32, tag=f"sc{h_in}")
                            for j in range(2):
                                tt = tt2 * 2 + j
                                nc.tensor.matmul(
                                    sc[:, j, :],
                                    (kT[p0:p0 + 64, hp, tt * 128:(tt + 1) * 128]),
                                    (qT[p0:p0 + 64, hp, ssl]),
                                    start=True,
                                    stop=True,
                                )
                            ex = expool.tile([128, 2, SBK], F32R, tag=f"ex{h_in}")
                            nc.scalar.activation(out=ex, in_=sc, func=AF.Exp)
                            for j in range(2):
                                tt = tt2 * 2 + j
                                nc.tensor.matmul(
                                    oaugs[h_in],
                                    (vaug[:, tt, h, :]),
                                    (ex[:, j, :]),
                                    start=(tt == 0),
                                    stop=(tt == NTT - 1),
                                )
                    for h_in in range(2):
                        p0 = h_in * 64
                        recip32 = npool.tile([1, SBK], F32, tag="recip32")
                        nc.vector.reciprocal(out=recip32, in_=oaugs[h_in][D:D + 1, :])
                        recip = npool.tile([1, SBK], F32R, tag="recip")
                        nc.vector.tensor_copy(out=recip, in_=recip32)
                        bc = bcpool.tile([D, SBK], F32, tag="bc")
                        nc.tensor.matmul(
                            bc, (ones_sb), (recip), start=True, stop=True
                        )
                        rb = npool.tile([D, SBK], F32, tag="rb")
                        nc.vector.tensor_copy(out=rb, in_=bc)
                        nc.vector.tensor_mul(
                            out=aoT[p0:p0 + 64, hp, ssl],
                            in0=oaugs[h_in][0:D, :],
                            in1=rb,
                        )

        # ---------------- Phase C: output projection (partial Wo) ----------------
        with tc.tile_pool(name="wos", bufs=1) as wopool, \
             tc.tile_pool(name="wops", bufs=2, space="PSUM") as wpp, \
             tc.tile_pool(name="outs", bufs=3) as outpool:
            wo_sb = wopool.tile([128, NM, E // 512, 512], F32R)
            bo_sb = wopool.tile([128, E], F32)
            nc.sync.dma_start(out=wo_sb, in_=wo)
            nc.sync.dma_start(out=bo_sb, in_=bo)
            for mt in range(MT):
                for nb2 in range(E // 512):
                    ps = wpp.tile([128, 512], F32, tag="wops")
                    for kc in range(NM):
                        nc.tensor.matmul(
                            ps,
                            (aoT[:, kc, mt * 128:(mt + 1) * 128]),
                            (wo_sb[:, kc, nb2, :]),
                            start=(kc == 0),
                            stop=(kc == NM - 1),
                        )
                    ot = outpool.tile([128, 512], F32, tag="ot")
                    nc.vector.tensor_add(
                        out=ot, in0=ps, in1=bo_sb[:, nb2 * 512:(nb2 + 1) * 512]
                    )
                    nc.sync.dma_start(
                        out=out[mt * 128:(mt + 1) * 128, nb2 * 512:(nb2 + 1) * 512],
                        in_=ot,
                    )

    nc.compile()
    return nc


_PROG = {}


def _get_prog(S=2048, num_devices=8):
    key = (S, num_devices)
    if key not in _PROG:
        _PROG[key] = build_program(S, num_devices)
    return _PROG[key]


def _tile_x(x2d):
    # [S, E] slice -> [128, KT, S] with element (p, k, t) = x2d[t, k*128+p]
    S = x2d.shape[0]
    xt = np.ascontiguousarray(x2d.T.astype(np.float32))
    return np.ascontiguousarray(xt.reshape(KT, 128, S).transpose(1, 0, 2))


def _tile_w(weff, ch0):
    w = weff[:, ch0:ch0 + CH]
    return np.ascontiguousarray(
        w.reshape(KT, 128, CH).transpose(1, 0, 2).astype(np.float32)
    )


def prep_in_maps(x_q, x_k, x_v, Wq, bq, Aq, Bq, Wk, bk, Wv, bv, Av, Bv, Wo, bo):
    x_q = np.asarray(x_q, np.float32)
    x_k = np.asarray(x_k, np.float32)
    x_v = np.asarray(x_v, np.float32)
    scaling = 2.0  # lora_alpha / r = 32 / 16
    wq_eff = (np.asarray(Wq).T + (np.asarray(Aq) @ np.asarray(Bq)) * scaling).astype(
        np.float32
    )
    wv_eff = (np.asarray(Wv).T + (np.asarray(Av) @ np.asarray(Bv)) * scaling).astype(
        np.float32
    )
    wk_s = (np.asarray(Wk).T / 8.0).astype(np.float32)  # sqrt(D) folded in
    bk_s = (np.asarray(bk) / 8.0).astype(np.float32)
    bq = np.asarray(bq, np.float32)
    bv = np.asarray(bv, np.float32)
    bo = np.asarray(bo, np.float32)
    woT = np.ascontiguousarray(np.asarray(Wo).T.astype(np.float32))

    nbatch = x_q.shape[1]
    in_maps = []
    for c in range(2 * nbatch):
        b = c // 2
        hg = c % 2
        ch0 = hg * CH
        wo_c = np.ascontiguousarray(
            woT[ch0:ch0 + CH, :].reshape(CH // 128, 128, E // 512, 512)
            .transpose(1, 0, 2, 3)
        )
        in_maps.append({
            "xq": _tile_x(x_q[:, b, :]),
            "xk": _tile_x(x_k[:, b, :]),
            "xv": _tile_x(x_v[:, b, :]),
            "wq": _tile_w(wq_eff, ch0),
            "wk": _tile_w(wk_s, ch0),
            "wv": _tile_w(wv_eff, ch0),
            "wo": wo_c,
            "bq": np.ascontiguousarray(bq[ch0:ch0 + CH].reshape(CH // 128, 128).T),
            "bk": np.ascontiguousarray(bk_s[ch0:ch0 + CH].reshape(CH // 128, 128).T),
            "bv": np.ascontiguousarray(np.broadcast_to(bv[ch0:ch0 + CH], (128, CH))),
            "onesd": np.ones(64, np.float32),
            "bo": (
                np.ascontiguousarray(np.broadcast_to(bo, (128, E)))
                if hg == 0
                else np.zeros((128, E), np.float32)
            ),
        })
    return in_maps


def gather_out(results, nbatch):
    return np.stack(
        [results[2 * b]["out"] + results[2 * b + 1]["out"] for b in range(nbatch)],
        axis=1,
    )


def kernel(**inputs):
    nc = _get_prog(2048, 8)
    in_maps = prep_in_maps(**inputs)
    res = run_bass_kernel_spmd(nc, in_maps, core_ids=list(range(NCORES)))
    return gather_out(res.results, B)



# revision 25
# speedup vs baseline: 1.3176x; 1.3176x over previous
"""LoRA multi-head attention kernel for 8 Trainium2 NeuronCores.

Problem: q = x_q@(Wq.T + Aq@Bq*2) + bq ; k = x_k@Wk.T + bk ;
         v = x_v@(Wv.T + Av@Bv*2) + bv ; MHA over 16 heads, D=64,
         out = attn_out @ Wo.T + bo.   Shapes: x [2048, 4, 1024].

Sharding: core c handles batch b = c//2 and head-group hg = c%2
(8 heads = 512 channels). LoRA weights are merged on the host
(mathematically exact), the 1/sqrt(D) score scale is folded into Wk/bk,
and x is transposed on the host so every matmul contracts over the
partition dimension. Each core computes a partial output
(its 512 channels through Wo); the host sums the two partials per batch.

Device pipeline per core (all matmul inputs bf16 except scores in fp8):
  - q/k projections write fp8e4m3 qT/kT [ch, tok] with a zeroed second
    "DoubleRow plane"; the QK^T matmuls then run in fp8 DoubleRow mode
    (plane 1 contributes exactly zero), at half PE cost.
  - exp runs on the Activation engine out of a 6-bank PSUM score buffer
    in 3-bank groups, writing bf16 ex tiles.
  - attn@v is oriented [s_tile=128, D+1] (ex stationary, [v|1] moving)
    so all 128 output partitions are used; column D is the softmax
    denominator; a single DVE divide normalizes, a PE transpose flips
    back to [ch, tok] for the bf16 output projection.
  - phase work is software-pipelined so the Activation engine (the
    serial-exp floor) runs continuously: q/k chunks 1-3 and the v
    projection are emitted as filler between attention units.
"""

import sys

import numpy as np

sys.path.insert(0, "/opt/trn_rl_repo")

from contextlib import ExitStack  # noqa: E402

import ml_dtypes  # noqa: E402

import concourse.bass as bass  # noqa: E402
import concourse.tile as tile  # noqa: E402
from concourse import bacc, mybir  # noqa: E402
from concourse.bass_utils import run_bass_kernel_spmd  # noqa: E402

F32 = mybir.dt.float32
BF16 = mybir.dt.bfloat16
F8 = mybir.dt.float8e4
AF = mybir.ActivationFunctionType
ALU = mybir.AluOpType
DR = mybir.MatmulPerfMode.DoubleRow

E = 1024
D = 64
NHC = 8            # heads per core
CH = NHC * D       # 512 output channels per core
KT = E // 128      # k-tiles over the E contraction
NCORES = 8
B = 4
BF = ml_dtypes.bfloat16


TUNE = {
    "warmup": 16,
    "b_early": 8.4,
    "b_mid": 5.5,
    "b_late": 6.0,
    "cost_v": 4.2,
    "cost_qk": 2.2,
    "cost_c": 2.2,
    "cbudget": 4.4,
    "early_units": 8,
    "mid_units": 24,
}


def build_program(S=2048, num_devices=8):
    TB = 256                        # token block for projections
    NTB = S // TB                   # 8
    NTT = S // 128                  # 16 t/tok tiles
    NSB = S // 512                  # 4 s-blocks
    NM = CH // 128                  # 4 ch chunks per core

    nc = bacc.Bacc(
        "TRN2", target_bir_lowering=False, debug=False, num_devices=num_devices
    )

    def dram(name, shape, out=False, dt=F32):
        kind = "ExternalOutput" if out else "ExternalInput"
        return nc.dram_tensor(name, shape, dt, kind=kind).ap()

    xq = dram("xq", [128, KT, S], dt=BF16)
    xk = dram("xk", [128, KT, S], dt=BF16)
    xv = dram("xv", [128, KT, S], dt=BF16)
    wq = dram("wq", [128, KT, CH], dt=BF16)
    wk = dram("wk", [128, KT, CH], dt=BF16)
    wv = dram("wv", [128, KT, CH], dt=BF16)
    wo = dram("wo", [128, NM, E // 512, 512], dt=BF16)
    bq = dram("bq", [128, NM])
    bk = dram("bk", [128, NM])
    bv = dram("bv", [128, CH], dt=BF16)
    bo = dram("bo", [128, E], dt=BF16)
    out = dram("out", [S, E], out=True, dt=BF16)

    with tile.TileContext(nc) as tc, ExitStack() as top:
        persist = top.enter_context(tc.tile_pool(name="persist", bufs=1))
        q8 = persist.tile([128, NM, S], F8)         # [ch%128, ch//128, tok]
        k8 = persist.tile([128, NM, S], F8)
        # innermost padded to 72 so the DoubleRow plane stride (8*72)
        # is a multiple of 64 (walrus ISA check on ldweights)
        vaug = persist.tile([128, NTT, NHC, 72], F8)   # [tok%128, tt, h, d|1]
        ones_sb = persist.tile([1, D], BF16)
        negone = persist.tile([128, 1], F32)
        aoTs = [persist.tile([128, NM, 512], BF16, name=f"aoT{s}") for s in range(NSB)]
        wv_sb = persist.tile([128, KT, CH], BF16)
        wo_sb = persist.tile([128, NM, E // 512, 512], BF16)
        bq_sb = persist.tile([128, NM], F32)
        bk_sb = persist.tile([128, NM], F32)
        bv_sb = persist.tile([128, CH], BF16)
        bo_sb = persist.tile([128, E], BF16)

        # DMA queue split: SP carries wk+xk (k path), the Activation
        # sequencer carries wq+xq then wv+xv+wo (q/v path); both x tensors
        # load as single full-tensor DMAs (few descriptors, early start).
        nc.sync.dma_start(out=bk_sb, in_=bk)
        nc.scalar.dma_start(out=bq_sb, in_=bq)
        warm = persist.tile([1, 256], BF16)
        nc.gpsimd.memset(warm, 1.0)
        nc.gpsimd.memset(vaug[:, :, :, D:D + 1], 1.0)
        nc.gpsimd.memset(ones_sb, 1.0)
        nc.gpsimd.memset(negone, -1.0)

        with tc.tile_pool(name="wqk", bufs=1) as wqk, \
             tc.tile_pool(name="xvs", bufs=4) as xvpool, \
             tc.tile_pool(name="exs", bufs=4) as expool, \
             tc.tile_pool(name="nrm", bufs=2) as npool, \
             tc.tile_pool(name="stg", bufs=2) as stgpool, \
             tc.tile_pool(name="pps", bufs=2, space="PSUM") as ppool:
            wq_sb = wqk.tile([128, KT, CH], BF16, tag="wq")
            wk_sb = wqk.tile([128, KT, CH], BF16, tag="wk")
            xq_sb = wqk.tile([128, KT, S], BF16, tag="xq")
            xk_sb = wqk.tile([128, KT, S], BF16, tag="xk")
            # critical-path loads in strict serial order on the SP queue:
            # only what unit 0 needs (k path fully, q for s-block 0)
            nc.sync.dma_start(out=wk_sb[:, :, 0:128], in_=wk[:, :, 0:128])
            nc.sync.dma_start(out=wq_sb[:, :, 0:128], in_=wq[:, :, 0:128])
            def _ld(dst, srct, nb):
                sl = slice(nb * TB, (nb + 1) * TB)
                nc.sync.dma_start(out=dst[:, :, sl], in_=srct[:, :, sl])

            def _xvld(nb):
                xt = xvpool.tile([128, KT, TB], BF16, tag="xv", name="xvt")
                nc.sync.dma_start(out=xt, in_=xv[:, :, nb * TB:(nb + 1) * TB])
                xv_tiles.append(xt)

            # one deterministic order on the (serial) DMA path, sorted by
            # consumer deadline; xv4-7 are slot-gated behind v consumption
            xv_tiles = []
            for nb in range(2):
                _ld(xk_sb, xk, nb)
            for nb in range(2):
                _ld(xq_sb, xq, nb)
            for nb in range(2, NTB):
                _ld(xk_sb, xk, nb)
            nc.sync.dma_start(out=wv_sb, in_=wv)
            _xvld(0)
            _xvld(1)
            _ld(xq_sb, xq, 2)
            _ld(xq_sb, xq, 3)
            _xvld(2)
            _xvld(3)
            _ld(xq_sb, xq, 4)
            _ld(xq_sb, xq, 5)
            nc.sync.dma_start(out=bv_sb, in_=bv)
            _xvld(4)
            _xvld(5)
            _ld(xq_sb, xq, 6)
            _ld(xq_sb, xq, 7)
            _xvld(6)
            _xvld(7)
            nc.sync.dma_start(
                out=wk_sb[:, :, 128:CH], in_=wk[:, :, 128:CH]
            )
            nc.sync.dma_start(
                out=wq_sb[:, :, 128:CH], in_=wq[:, :, 128:CH]
            )
            nc.sync.dma_start(out=bo_sb, in_=bo)

            # ---- emission tasks (spread between attention units) ----------

            def qk_task(which, hp, nb):
                """Project one [128ch, 256tok] chunk of q or k into fp8."""
                x_sb, w_sb, b_sb, dst = (
                    (xk_sb, wk_sb, bk_sb, k8)
                    if which == "k"
                    else (xq_sb, wq_sb, bq_sb, q8)
                )
                pq = ppool.tile([128, 512], F32, tag="pp", name="pq")
                for k in range(KT):
                    nc.tensor.matmul(
                        pq[:, 0:TB],
                        (w_sb[:, k, hp * 128:(hp + 1) * 128]),
                        (x_sb[:, k, nb * TB:(nb + 1) * TB]),
                        start=(k == 0),
                        stop=(k == KT - 1),
                    )
                nc.vector.tensor_scalar(
                    out=dst[:, hp, nb * TB:(nb + 1) * TB],
                    in0=pq[:, 0:TB],
                    scalar1=b_sb[:, hp:hp + 1],
                    scalar2=None,
                    op0=ALU.add,
                )

            def v_task(nb):
                """Project one 256-token block of v into fp8 vaug."""
                xt = xv_tiles[nb]
                for mi in range(TB // 128):
                    tt = nb * (TB // 128) + mi
                    pv = ppool.tile([128, 512], F32, tag="pp", name="pv")
                    for k in range(KT):
                        nc.tensor.matmul(
                            pv,
                            (xt[:, k, mi * 128:(mi + 1) * 128]),
                            (wv_sb[:, k, :]),
                            start=(k == 0),
                            stop=(k == KT - 1),
                        )
                    nc.vector.tensor_add(
                        out=vaug[:, tt, :, 0:D],
                        in0=pv.rearrange("p (h d) -> p h d", d=D),
                        in1=bv_sb.rearrange("p (h d) -> p h d", d=D),
                    )

            done_sb = {}        # sb -> finished (hp, h_in) units

            def av_task(hp, sb, h_in, ex):
                """attn@v (fp8 DoubleRow, two t-tiles per matmul) + softmax
                normalize for one head."""
                h = 2 * hp + h_in
                p0 = h_in * 64
                oa = ppool.tile([128, 512], F32, tag="pp", name="oa")
                for p in range(NTT // 2):
                    nc.tensor.matmul(
                        oa[0:D + 1, :],
                        (vaug[:, 2 * p:2 * p + 2, h, 0:D + 1]),
                        (ex[:, 2 * p:2 * p + 2, :]),
                        start=(p == 0),
                        stop=(p == NTT // 2 - 1),
                        perf_mode=DR,
                    )
                recip = npool.tile([1, 512], F32, tag="rc", name="recip")
                nc.vector.reciprocal(out=recip, in_=oa[D:D + 1, :])
                rb = npool.tile([D, 512], F32, tag="rbf", name="rb")
                nc.gpsimd.partition_broadcast(rb, recip)
                nc.vector.tensor_mul(
                    out=aoTs[sb][p0:p0 + 64, hp, :], in0=oa[0:D, :], in1=rb
                )
                done_sb.setdefault(sb, set()).add((hp, h_in))
                return sb if len(done_sb[sb]) == 2 * NM else None

            def c_task(sb, st):
                """Output projection for one 128-token tile."""
                stg = stgpool.tile([128, 2, 512], BF16, tag="stg", name="stg")
                for nb2 in range(E // 512):
                    pc = ppool.tile([128, 512], F32, tag="pp", name="pc")
                    for kc in range(NM):
                        nc.tensor.matmul(
                            pc,
                            (aoTs[sb][:, kc, st * 128:(st + 1) * 128]),
                            (wo_sb[:, kc, nb2, :]),
                            start=(kc == 0),
                            stop=(kc == NM - 1),
                        )
                    nc.vector.tensor_add(
                        out=stg[:, nb2, :],
                        in0=pc,
                        in1=bo_sb[:, nb2 * 512:(nb2 + 1) * 512],
                    )
                mt = sb * 4 + st
                nc.gpsimd.dma_start(
                    out=out[mt * 128:(mt + 1) * 128, :],
                    in_=stg.rearrange("p a b -> p (a b)"),
                )

            # PE warmup: junk matmuls ramp the tensor engine to its full
            # clock (3us of continuous busy) while the first x chunks load
            wps = ppool.tile([128, 512], F32, tag="pp", name="wps")
            for i in range(TUNE["warmup"]):
                nc.tensor.matmul(
                    wps[0:64, 0:256], (warm[:, 0:D]), (warm),
                    start=True, stop=True,
                )
            # chunk 0 of q/k: only the k/q blocks unit 0 needs are run
            # up front; the rest interleave with unit 0's score groups
            # (score group g only reads k tokens up to tile 3g+2).
            for nb in (0, 1):
                qk_task("k", 0, nb)
            for nb in (0, 1):
                qk_task("q", 0, nb)
            qk0_pending = [("k", 0, nb) for nb in range(2, NTB)]

            # fill queue: v interleaved with qk chunk 1, then chunks 2-3.
            # costs are approximate PE microseconds, used to budget how much
            # filler is emitted per attention unit so the Act engine (exp)
            # never starves.
            fill = [("v", nb, TUNE["cost_v"]) for nb in range(NTB)]
            for hp in range(1, NM):
                fill += [("k", hp, nb, TUNE["cost_qk"]) for nb in range(NTB)]
                fill += [("q", hp, nb, TUNE["cost_qk"]) for nb in range(NTB)]
            fill += [("q", 0, nb, TUNE["cost_qk"]) for nb in range(2, NTB)]
            v_left = NTB

            units = [
                (hp, sb, h_in)
                for hp in range(NM)
                for sb in range(NSB)
                for h_in in range(2)
            ]
            groups = [(0, 3), (3, 3), (6, 3), (9, 3), (12, 3), (15, 1)]
            av_queue = []
            c_queue = []

            def run_fill(t):
                nonlocal v_left
                if t[0] == "v":
                    v_task(t[1])
                    v_left -= 1
                else:
                    qk_task(t[0], t[1], t[2])

            for ui, (hp, sb, h_in) in enumerate(units):
                if ui == 8:
                    nc.scalar.dma_start(out=wo_sb, in_=wo)
                # force projection work this unit depends on: all k blocks
                # of chunk hp, q blocks for this s-block only
                if ui > 0:
                    for i in reversed([
                        i for i, t in enumerate(fill)
                        if (t[0] == "k" and t[1] == hp)
                        or (t[0] == "q" and t[1] == hp
                            and t[2] in (2 * sb, 2 * sb + 1))
                    ]):
                        run_fill(fill.pop(i))
                p0 = h_in * 64
                ex = expool.tile([128, NTT, 512], F8, tag="ex", name="ex")
                for g0, glen in groups:
                    if ui == 0:
                        # feed k-chunk tasks just ahead of the t-tiles the
                        # next score group reads (DMA-paced startup)
                        need_nb = min((g0 + glen + 2) // 2, NTB - 1)
                        while qk0_pending and qk0_pending[0][2] <= need_nb:
                            qk_task(*qk0_pending.pop(0))
                    scp = ppool.tile([128, 3, 512], F32, tag="sc", name="scp")
                    for j in range(glen):
                        tt = g0 + j
                        nc.tensor.matmul(
                            scp[:, j, :],
                            (k8[p0:p0 + 64, hp, tt * 128:(tt + 1) * 128]
                             .unsqueeze(1).broadcast_to([64, 2, 128])),
                            (q8[p0:p0 + 64, hp, sb * 512:(sb + 1) * 512]
                             .unsqueeze(1).broadcast_to([64, 2, 512])),
                            start=True,
                            stop=True,
                            perf_mode=DR,
                        )
                    nc.scalar.activation(
                        out=ex[:, g0:g0 + glen, :],
                        in_=scp[:, 0:glen, :],
                        func=AF.Exp,
                        bias=negone,
                    )
                if ui == 0:
                    while qk0_pending:
                        qk_task(*qk0_pending.pop(0))
                av_queue.append((hp, sb, h_in, ex))
                # attn@v keeps pace with exp (self-limited by ex buffers);
                # budgeted filler tops up PE without starving the Act engine
                while av_queue and v_left == 0:
                    t = av_queue.pop(0)
                    full = av_task(*t)
                    if full is not None:
                        c_queue += [(full, st) for st in range(4)]
                budget = (TUNE["b_early"] if ui < TUNE["early_units"]
                          else TUNE["b_mid"] if ui < TUNE["mid_units"]
                          else TUNE["b_late"])
                cbudget = TUNE["cbudget"]
                while budget > 0:
                    if fill:
                        t = fill.pop(0)
                        run_fill(t)
                        budget -= t[-1]
                    elif c_queue and cbudget > 0:
                        c_task(*c_queue.pop(0))
                        budget -= TUNE["cost_c"]
                        cbudget -= TUNE["cost_c"]
                    else:
                        break

            while fill:
                run_fill(fill.pop(0))
            while av_queue:
                t = av_queue.pop(0)
                full = av_task(*t)
                if full is not None:
                    c_queue += [(full, st) for st in range(4)]
            while c_queue:
                c_task(*c_queue.pop(0))

    nc.compile()
    return nc


_PROG = {}


def _get_prog(S=2048, num_devices=8):
    key = (S, num_devices)
    if key not in _PROG:
        _PROG[key] = build_program(S, num_devices)
    return _PROG[key]


def _tile_x(x2d):
    # [S, E] slice -> [128, KT, S] with element (p, k, t) = x2d[t, k*128+p]
    S = x2d.shape[0]
    xt = np.ascontiguousarray(x2d.T)
    return np.ascontiguousarray(
        xt.reshape(KT, 128, S).transpose(1, 0, 2).astype(BF)
    )


def _tile_w(weff, ch0):
    w = weff[:, ch0:ch0 + CH]
    return np.ascontiguousarray(
        w.reshape(KT, 128, CH).transpose(1, 0, 2).astype(BF)
    )


def prep_in_maps(x_q, x_k, x_v, Wq, bq, Aq, Bq, Wk, bk, Wv, bv, Av, Bv, Wo, bo):
    x_q = np.asarray(x_q, np.float32)
    x_k = np.asarray(x_k, np.float32)
    x_v = np.asarray(x_v, np.float32)
    scaling = 2.0  # lora_alpha / r = 32 / 16
    wq_eff = (
        (np.asarray(Wq).T + (np.asarray(Aq) @ np.asarray(Bq)) * scaling) * 0.25
    ).astype(np.float32)
    wv_eff = (np.asarray(Wv).T + (np.asarray(Av) @ np.asarray(Bv)) * scaling).astype(
        np.float32
    )
    wk_s = (np.asarray(Wk).T / 4.0).astype(np.float32)
    bk_s = (np.asarray(bk) / 4.0).astype(np.float32)
    bq = np.asarray(bq, np.float32) * 0.25
    bv = np.asarray(bv, np.float32)
    bo = np.asarray(bo, np.float32)
    woT = np.ascontiguousarray(np.asarray(Wo).T.astype(np.float32))
    identity = np.eye(128, dtype=BF)

    nbatch = x_q.shape[1]
    in_maps = []
    for c in range(2 * nbatch):
        b = c // 2
        hg = c % 2
        ch0 = hg * CH
        wo_c = np.ascontiguousarray(
            woT[ch0:ch0 + CH, :].reshape(CH // 128, 128, E // 512, 512)
            .transpose(1, 0, 2, 3).astype(BF)
        )
        in_maps.append({
            "xq": _tile_x(x_q[:, b, :]),
            "xk": _tile_x(x_k[:, b, :]),
            "xv": _tile_x(x_v[:, b, :]),
            "wq": _tile_w(wq_eff, ch0),
            "wk": _tile_w(wk_s, ch0),
            "wv": _tile_w(wv_eff, ch0),
            "wo": wo_c,
            "bq": np.ascontiguousarray(bq[ch0:ch0 + CH].reshape(CH // 128, 128).T),
            "bk": np.ascontiguousarray(bk_s[ch0:ch0 + CH].reshape(CH // 128, 128).T),
            "bv": np.ascontiguousarray(np.broadcast_to(bv[ch0:ch0 + CH], (128, CH)).astype(BF)),
            "ident": identity,
            "bo": (
                np.ascontiguousarray(np.broadcast_to(bo, (128, E)).astype(BF))
                if hg == 0
                else np.zeros((128, E), BF)
            ),
        })
    return in_maps


def gather_out(results, nbatch):
    return np.stack(
        [
            results[2 * b]["out"].astype(np.float32)
            + results[2 * b + 1]["out"].astype(np.float32)
            for b in range(nbatch)
        ],
        axis=1,
    )


def kernel(**inputs):
    nc = _get_prog(2048, 8)
    in_maps = prep_in_maps(**inputs)
    res = run_bass_kernel_spmd(nc, in_maps, core_ids=list(range(NCORES)))
    return gather_out(res.results, B)


# revision 33
# speedup vs baseline: 1.4712x; 1.1166x over previous
"""LoRA multi-head attention kernel for 8 Trainium2 NeuronCores.

Problem: q = x_q@(Wq.T + Aq@Bq*2) + bq ; k = x_k@Wk.T + bk ;
         v = x_v@(Wv.T + Av@Bv*2) + bv ; MHA over 16 heads, D=64,
         out = attn_out @ Wo.T + bo.   Shapes: x [2048, 4, 1024].

Sharding: core c handles batch b = c//2 and head-group hg = c%2
(8 heads = 512 channels). LoRA weights are merged on the host
(mathematically exact), the 1/sqrt(D) score scale is folded into Wk/bk,
and x is transposed on the host so every matmul contracts over the
partition dimension. Each core computes a partial output
(its 512 channels through Wo); the host sums the two partials per batch.

Device pipeline per core (all matmul inputs bf16 except scores in fp8):
  - q/k projections write fp8e4m3 qT/kT [ch, tok] with a zeroed second
    "DoubleRow plane"; the QK^T matmuls then run in fp8 DoubleRow mode
    (plane 1 contributes exactly zero), at half PE cost.
  - exp runs on the Activation engine out of a 6-bank PSUM score buffer
    in 3-bank groups, writing bf16 ex tiles.
  - attn@v is oriented [s_tile=128, D+1] (ex stationary, [v|1] moving)
    so all 128 output partitions are used; column D is the softmax
    denominator; a single DVE divide normalizes, a PE transpose flips
    back to [ch, tok] for the bf16 output projection.
  - phase work is software-pipelined so the Activation engine (the
    serial-exp floor) runs continuously: q/k chunks 1-3 and the v
    projection are emitted as filler between attention units.
"""

import sys

import numpy as np

sys.path.insert(0, "/opt/trn_rl_repo")

from contextlib import ExitStack  # noqa: E402

import ml_dtypes  # noqa: E402

import concourse.bass as bass  # noqa: E402
import concourse.tile as tile  # noqa: E402
from concourse import bacc, mybir  # noqa: E402
from concourse.bass_utils import run_bass_kernel_spmd  # noqa: E402

F32 = mybir.dt.float32
BF16 = mybir.dt.bfloat16
F8 = mybir.dt.float8e4
AF = mybir.ActivationFunctionType
ALU = mybir.AluOpType
DR = mybir.MatmulPerfMode.DoubleRow

E = 1024
D = 64
NHC = 8            # heads per core
CH = NHC * D       # 512 output channels per core
KT = E // 128      # k-tiles over the E contraction
NCORES = 8
B = 4
BF = ml_dtypes.bfloat16


TUNE = {
    "warmup": 16,
    "b_early": 8.4,
    "b_mid": 4.4,
    "b_late": 4.4,
    "cost_v": 5.2,
    "cost_qk": 3.2,
    "cost_c": 3.0,
    "cbudget": 4.4,
    "early_units": 8,
    "mid_units": 24,
}


def build_program(S=2048, num_devices=8):
    TB = 256                        # token block for projections
    NTB = S // TB                   # 8
    NTT = S // 128                  # 16 t/tok tiles
    NSB = S // 512                  # 4 s-blocks
    NM = CH // 128                  # 4 ch chunks per core

    nc = bacc.Bacc(
        "TRN2", target_bir_lowering=False, debug=False, num_devices=num_devices
    )

    def dram(name, shape, out=False, dt=F32):
        kind = "ExternalOutput" if out else "ExternalInput"
        return nc.dram_tensor(name, shape, dt, kind=kind).ap()

    xq = dram("xq", [128, KT, S], dt=BF16)
    xk = dram("xk", [128, KT, S], dt=BF16)
    xv = dram("xv", [128, KT, S], dt=BF16)
    wq = dram("wq", [128, KT, CH], dt=BF16)
    wk = dram("wk", [128, KT, CH], dt=BF16)
    wv = dram("wv", [128, KT, CH], dt=BF16)
    wo = dram("wo", [128, NM, E // 512, 512], dt=BF16)
    bq = dram("bq", [128, NM])
    bk = dram("bk", [128, NM])
    bv = dram("bv", [128, CH], dt=BF16)
    bo = dram("bo", [128, E], dt=BF16)
    out = dram("out", [S, E], out=True, dt=BF16)

    with tile.TileContext(nc) as tc, ExitStack() as top:
        persist = top.enter_context(tc.tile_pool(name="persist", bufs=1))
        q8 = persist.tile([128, NM, S], F8)         # [ch%128, ch//128, tok]
        k8 = persist.tile([128, NM, S], F8)
        # innermost padded to 72 so the DoubleRow plane stride (8*72)
        # is a multiple of 64 (walrus ISA check on ldweights)
        vaug = persist.tile([128, NTT, NHC, 72], F8)   # [tok%128, tt, h, d|1]
        ones_sb = persist.tile([1, D], BF16)
        negone = persist.tile([128, 1], F32)
        ex0 = persist.tile([128, 16, 512], F8)
        aoTs = [persist.tile([128, NM, 512], BF16, name=f"aoT{s}") for s in range(NSB)]
        wv_sb = persist.tile([128, KT, CH], BF16)
        wo_sb = persist.tile([128, NM, E // 512, 512], BF16)
        bq_sb = persist.tile([128, NM], F32)
        bk_sb = persist.tile([128, NM], F32)
        bv_sb = persist.tile([128, CH], BF16)
        bo_sb = persist.tile([128, E], BF16)

        # DMA queue split: SP carries wk+xk (k path), the Activation
        # sequencer carries wq+xq then wv+xv+wo (q/v path); both x tensors
        # load as single full-tensor DMAs (few descriptors, early start).
        nc.sync.dma_start(out=bk_sb, in_=bk)
        nc.scalar.dma_start(out=bq_sb, in_=bq)
        warm = persist.tile([1, 256], BF16)
        nc.gpsimd.memset(warm, 1.0)
        nc.gpsimd.memset(vaug[:, :, :, D:D + 1], 1.0)
        nc.gpsimd.memset(ones_sb, 1.0)
        nc.gpsimd.memset(negone, -1.0)

        with tc.tile_pool(name="wqk", bufs=1) as wqk, \
             tc.tile_pool(name="xvs", bufs=4) as xvpool, \
             tc.tile_pool(name="pps", bufs=2, space="PSUM") as ppool:
            wq_sb = wqk.tile([128, KT, CH], BF16, tag="wq")
            wk_sb = wqk.tile([128, KT, CH], BF16, tag="wk")
            xq_sb = wqk.tile([128, KT, S], BF16, tag="xq")

            def _ld(dst, srct, nb):
                sl = slice(nb * TB, (nb + 1) * TB)
                nc.sync.dma_start(out=dst[:, :, sl], in_=srct[:, :, sl])

            def _xvld(nb):
                xt = xvpool.tile([128, KT, TB], BF16, tag="xv", name="xvt")
                nc.sync.dma_start(out=xt, in_=xv[:, :, nb * TB:(nb + 1) * TB])
                xv_tiles.append(xt)

            xv_tiles = []
            xk_src = {}

            def qk_task(which, hp, nb):
                """Project one [128ch, 256tok] chunk of q or k into fp8."""
                if which == "k":
                    w_sb, b_sb, dst = wk_sb, bk_sb, k8
                    if (hp, nb) in xk_src:
                        xs = xk_src.pop((hp, nb))
                        xsl = slice(nb * TB, (nb + 1) * TB)
                    else:
                        xs = xkpool.tile(
                            [128, KT, TB], BF16, tag="xk", name="xkt"
                        )
                        nc.sync.dma_start(
                            out=xs, in_=xk[:, :, nb * TB:(nb + 1) * TB]
                        )
                        xsl = slice(0, TB)
                else:
                    w_sb, b_sb, dst = wq_sb, bq_sb, q8
                    xs = xq_sb
                    xsl = slice(nb * TB, (nb + 1) * TB)
                pq = ppool.tile([128, 512], F32, tag="pp", name="pq")
                for k in range(KT):
                    nc.tensor.matmul(
                        pq[:, 0:TB],
                        (w_sb[:, k, hp * 128:(hp + 1) * 128]),
                        (xs[:, k, xsl]),
                        start=(k == 0),
                        stop=(k == KT - 1),
                    )
                nc.vector.tensor_scalar(
                    out=dst[:, hp, nb * TB:(nb + 1) * TB],
                    in0=pq[:, 0:TB],
                    scalar1=b_sb[:, hp:hp + 1],
                    scalar2=None,
                    op0=ALU.add,
                )

            def v_task(nb, mi):
                """Project one 128-token half-block of v into fp8 vaug."""
                xt = xv_tiles[nb]
                tt = nb * (TB // 128) + mi
                pv = ppool.tile([128, 512], F32, tag="pp", name="pv")
                for k in range(KT):
                    nc.tensor.matmul(
                        pv,
                        (xt[:, k, mi * 128:(mi + 1) * 128]),
                        (wv_sb[:, k, :]),
                        start=(k == 0),
                        stop=(k == KT - 1),
                    )
                nc.vector.tensor_add(
                    out=vaug[:, tt, :, 0:D],
                    in0=pv.rearrange("p (h d) -> p h d", d=D),
                    in1=bv_sb.rearrange("p (h d) -> p h d", d=D),
                )

            done_sb = {}        # sb -> finished (hp, h_in) units

            def av_task(hp, sb, h_in, ex):
                """attn@v (fp8 DoubleRow, two t-tiles per matmul) + softmax
                normalize for one head."""
                h = 2 * hp + h_in
                p0 = h_in * 64
                oa = ppool.tile([128, 512], F32, tag="pp", name="oa")
                for p in range(NTT // 2):
                    nc.tensor.matmul(
                        oa[0:D + 1, :],
                        (vaug[:, 2 * p:2 * p + 2, h, 0:D + 1]),
                        (ex[:, 2 * p:2 * p + 2, :]),
                        start=(p == 0),
                        stop=(p == NTT // 2 - 1),
                        perf_mode=DR,
                    )
                recip = npool.tile([1, 512], F32, tag="rc", name="recip")
                nc.vector.reciprocal(out=recip, in_=oa[D:D + 1, :])
                rb = npool.tile([D, 512], F32, tag="rbf", name="rb")
                nc.gpsimd.partition_broadcast(rb, recip)
                nc.vector.tensor_mul(
                    out=aoTs[sb][p0:p0 + 64, hp, :], in0=oa[0:D, :], in1=rb
                )
                done_sb.setdefault(sb, set()).add((hp, h_in))
                return sb if len(done_sb[sb]) == 2 * NM else None

            def c_task(sb, st):
                """Output projection for one 128-token tile."""
                stg = stgpool.tile([128, 2, 512], BF16, tag="stg", name="stg")
                for nb2 in range(E // 512):
                    pc = ppool.tile([128, 512], F32, tag="pp", name="pc")
                    for kc in range(NM):
                        nc.tensor.matmul(
                            pc,
                            (aoTs[sb][:, kc, st * 128:(st + 1) * 128]),
                            (wo_sb[:, kc, nb2, :]),
                            start=(kc == 0),
                            stop=(kc == NM - 1),
                        )
                    nc.vector.tensor_add(
                        out=stg[:, nb2, :],
                        in0=pc,
                        in1=bo_sb[:, nb2 * 512:(nb2 + 1) * 512],
                    )
                mt = sb * 4 + st
                nc.gpsimd.dma_start(
                    out=out[mt * 128:(mt + 1) * 128, :],
                    in_=stg.rearrange("p a b -> p (a b)"),
                )

            # ---- startup: chunk 0 of q/k in a scoped pool whose 32KB is
            # reclaimed by the ex pool once the chunk-0 projections finish
            with tc.tile_pool(name="xk0", bufs=1) as xk0p:
                xk0_sb = xk0p.tile([128, KT, S], BF16, tag="xk0")
                # serial DMA order, sorted by consumer deadline
                nc.sync.dma_start(out=wk_sb[:, :, 0:128], in_=wk[:, :, 0:128])
                nc.sync.dma_start(out=wq_sb[:, :, 0:128], in_=wq[:, :, 0:128])
                for nb in range(2):
                    _ld(xk0_sb, xk, nb)
                for nb in range(2):
                    _ld(xq_sb, xq, nb)
                for nb in range(2, NTB):
                    _ld(xk0_sb, xk, nb)
                nc.sync.dma_start(out=wv_sb, in_=wv)
                _xvld(0)
                _xvld(1)
                _ld(xq_sb, xq, 2)
                _ld(xq_sb, xq, 3)
                _xvld(2)
                _xvld(3)
                _ld(xq_sb, xq, 4)
                _ld(xq_sb, xq, 5)
                nc.sync.dma_start(out=bv_sb, in_=bv)
                _xvld(4)
                _xvld(5)
                _ld(xq_sb, xq, 6)
                _ld(xq_sb, xq, 7)
                _xvld(6)
                _xvld(7)
                nc.sync.dma_start(
                    out=wk_sb[:, :, 128:CH], in_=wk[:, :, 128:CH]
                )
                nc.sync.dma_start(
                    out=wq_sb[:, :, 128:CH], in_=wq[:, :, 128:CH]
                )
                nc.sync.dma_start(out=bo_sb, in_=bo)

                # PE warmup: junk matmuls ramp the tensor engine clock while
                # the first x chunks stream in
                wps = ppool.tile([128, 512], F32, tag="pp", name="wps")
                for i in range(TUNE["warmup"]):
                    nc.tensor.matmul(
                        wps[0:64, 0:256], (warm[:, 0:D]), (warm),
                        start=True, stop=True,
                    )
                for nb in (0, 1):
                    xk_src[(0, nb)] = xk0_sb
                    qk_task("k", 0, nb)
                for nb in (0, 1):
                    qk_task("q", 0, nb)
                # unit 0 (hp0, sb0, h0) interleaved with the remaining
                # chunk-0 k-tasks: score group g only needs k tokens up to
                # tile 3g+2, so exp starts while k still streams in
                qk0_pending = list(range(2, NTB))
                for g0, glen in [(0, 3), (3, 3), (6, 3), (9, 3), (12, 3),
                                 (15, 1)]:
                    need_nb = min((g0 + glen + 2) // 2, NTB - 1)
                    while qk0_pending and qk0_pending[0] <= need_nb:
                        nb = qk0_pending.pop(0)
                        xk_src[(0, nb)] = xk0_sb
                        qk_task("k", 0, nb)
                    scp = ppool.tile([128, 3, 512], F32, tag="sc", name="scp")
                    for j in range(glen):
                        tt = g0 + j
                        nc.tensor.matmul(
                            scp[:, j, :],
                            (k8[0:64, 0, tt * 128:(tt + 1) * 128]
                             .unsqueeze(1).broadcast_to([64, 2, 128])),
                            (q8[0:64, 0, 0:512]
                             .unsqueeze(1).broadcast_to([64, 2, 512])),
                            start=True,
                            stop=True,
                            perf_mode=DR,
                        )
                    nc.scalar.activation(
                        out=ex0[:, g0:g0 + glen, :],
                        in_=scp[:, 0:glen, :],
                        func=AF.Exp,
                        bias=negone,
                    )
                for nb in qk0_pending:
                    xk_src[(0, nb)] = xk0_sb
                    qk_task("k", 0, nb)

            with tc.tile_pool(name="xks", bufs=3) as xkpool, \
                 tc.tile_pool(name="exs", bufs=5) as expool, \
                 tc.tile_pool(name="nrm", bufs=2) as npool, \
                 tc.tile_pool(name="stg", bufs=2) as stgpool:
                # fill queue: v interleaved with qk chunk 1, then chunks 2-3.
                # costs are approximate PE microseconds, used to budget how much
                # filler is emitted per attention unit so the Act engine (exp)
                # never starves.
                fill = [("v", (nb, mi), TUNE["cost_v"] / 2)
                        for nb in range(NTB) for mi in range(2)]
                for hp in range(1, NM):
                    fill += [("k", hp, nb, TUNE["cost_qk"]) for nb in range(NTB)]
                    fill += [("q", hp, nb, TUNE["cost_qk"]) for nb in range(NTB)]
                fill += [("q", 0, nb, TUNE["cost_qk"]) for nb in range(2, NTB)]
                v_left = 2 * NTB

                units = [
                    (hp, sb, h_in)
                    for hp in range(NM)
                    for sb in range(NSB)
                    for h_in in range(2)
                ]
                groups = [(0, 3), (3, 3), (6, 3), (9, 3), (12, 3), (15, 1)]
                av_queue = [(0, 0, 0, ex0)]
                c_queue = []

                def run_fill(t):
                    nonlocal v_left
                    if t[0] == "v":
                        v_task(*t[1])
                        v_left -= 1
                    else:
                        qk_task(t[0], t[1], t[2])

                budget_acc = [0.0]

                def emit_quanta(limit):
                    # attn@v keeps pace with exp (self-limited by ex bufs);
                    # small filler quanta keep PE fed without letting any
                    # single task starve the next score group
                    budget_acc[0] = min(budget_acc[0] + limit, 3.0 * limit)
                    while av_queue and v_left == 0:
                        t = av_queue.pop(0)
                        full = av_task(*t)
                        if full is not None:
                            c_queue.extend((full, st) for st in range(4))
                    while budget_acc[0] > 0:
                        if fill:
                            t = fill.pop(0)
                            run_fill(t)
                            budget_acc[0] -= t[-1]
                        elif c_queue:
                            c_task(*c_queue.pop(0))
                            budget_acc[0] -= TUNE["cost_c"]
                        else:
                            break

                for ui, (hp, sb, h_in) in enumerate(units):
                    if ui == 0:
                        continue  # unit 0 emitted inside the xk0 scope
                    if ui == 8:
                        nc.scalar.dma_start(out=wo_sb, in_=wo)
                    # force projection work this unit depends on: all k
                    # blocks of chunk hp, q blocks for this s-block only
                    for i in reversed([
                        i for i, t in enumerate(fill)
                        if (t[0] == "k" and t[1] == hp)
                        or (t[0] == "q" and t[1] == hp
                            and t[2] in (2 * sb, 2 * sb + 1))
                    ]):
                        run_fill(fill.pop(i))
                    per_group = (TUNE["b_early"] if ui < TUNE["early_units"]
                                 else TUNE["b_mid"] if ui < TUNE["mid_units"]
                                 else TUNE["b_late"]) / 6.0
                    if ui >= 28:
                        per_group = 2.0
                    p0 = h_in * 64
                    ex = expool.tile([128, NTT, 512], F8, tag="ex", name="ex")
                    for g0, glen in groups:
                        scp = ppool.tile([128, 3, 512], F32, tag="sc", name="scp")
                        for j in range(glen):
                            tt = g0 + j
                            nc.tensor.matmul(
                                scp[:, j, :],
                                (k8[p0:p0 + 64, hp, tt * 128:(tt + 1) * 128]
                                 .unsqueeze(1).broadcast_to([64, 2, 128])),
                                (q8[p0:p0 + 64, hp, sb * 512:(sb + 1) * 512]
                                 .unsqueeze(1).broadcast_to([64, 2, 512])),
                                start=True,
                                stop=True,
                                perf_mode=DR,
                            )
                        nc.scalar.activation(
                            out=ex[:, g0:g0 + glen, :],
                            in_=scp[:, 0:glen, :],
                            func=AF.Exp,
                            bias=negone,
                        )
                        emit_quanta(per_group)
                    av_queue.append((hp, sb, h_in, ex))

                while fill:
                    run_fill(fill.pop(0))
                while av_queue:
                    t = av_queue.pop(0)
                    full = av_task(*t)
                    if full is not None:
                        c_queue += [(full, st) for st in range(4)]
                while c_queue:
                    c_task(*c_queue.pop(0))

    nc.compile()
    return nc


_PROG = {}


def _get_prog(S=2048, num_devices=8):
    key = (S, num_devices)
    if key not in _PROG:
        _PROG[key] = build_program(S, num_devices)
    return _PROG[key]


def _tile_x(x2d):
    # [S, E] slice -> [128, KT, S] with element (p, k, t) = x2d[t, k*128+p]
    S = x2d.shape[0]
    xt = np.ascontiguousarray(x2d.T)
    return np.ascontiguousarray(
        xt.reshape(KT, 128, S).transpose(1, 0, 2).astype(BF)
    )


def _tile_w(weff, ch0):
    w = weff[:, ch0:ch0 + CH]
    return np.ascontiguousarray(
        w.reshape(KT, 128, CH).transpose(1, 0, 2).astype(BF)
    )


def prep_in_maps(x_q, x_k, x_v, Wq, bq, Aq, Bq, Wk, bk, Wv, bv, Av, Bv, Wo, bo):
    x_q = np.asarray(x_q, np.float32)
    x_k = np.asarray(x_k, np.float32)
    x_v = np.asarray(x_v, np.float32)
    scaling = 2.0  # lora_alpha / r = 32 / 16
    wq_eff = (
        (np.asarray(Wq).T + (np.asarray(Aq) @ np.asarray(Bq)) * scaling) * 0.25
    ).astype(np.float32)
    wv_eff = (np.asarray(Wv).T + (np.asarray(Av) @ np.asarray(Bv)) * scaling).astype(
        np.float32
    )
    wk_s = (np.asarray(Wk).T / 4.0).astype(np.float32)
    bk_s = (np.asarray(bk) / 4.0).astype(np.float32)
    bq = np.asarray(bq, np.float32) * 0.25
    bv = np.asarray(bv, np.float32)
    bo = np.asarray(bo, np.float32)
    woT = np.ascontiguousarray(np.asarray(Wo).T.astype(np.float32))
    identity = np.eye(128, dtype=BF)

    nbatch = x_q.shape[1]
    in_maps = []
    for c in range(2 * nbatch):
        b = c // 2
        hg = c % 2
        ch0 = hg * CH
        wo_c = np.ascontiguousarray(
                woT[ch0:ch0 + CH, :].reshape(CH // 128, 128, E // 512, 512)
                .transpose(1, 0, 2, 3).astype(BF)
        )
        in_maps.append({
                "xq": _tile_x(x_q[:, b, :]),
                "xk": _tile_x(x_k[:, b, :]),
                "xv": _tile_x(x_v[:, b, :]),
                "wq": _tile_w(wq_eff, ch0),
                "wk": _tile_w(wk_s, ch0),
                "wv": _tile_w(wv_eff, ch0),
                "wo": wo_c,
                "bq": np.ascontiguousarray(bq[ch0:ch0 + CH].reshape(CH // 128, 128).T),
                "bk": np.ascontiguousarray(bk_s[ch0:ch0 + CH].reshape(CH // 128, 128).T),
                "bv": np.ascontiguousarray(np.broadcast_to(bv[ch0:ch0 + CH], (128, CH)).astype(BF)),
                "ident": identity,
                "bo": (
                    np.ascontiguousarray(np.broadcast_to(bo, (128, E)).astype(BF))
                    if hg == 0
                    else np.zeros((128, E), BF)
                ),
        })
    return in_maps


def gather_out(results, nbatch):
    return np.stack(
        [
                results[2 * b]["out"].astype(np.float32)
                + results[2 * b + 1]["out"].astype(np.float32)
                for b in range(nbatch)
        ],
        axis=1,
    )


def kernel(**inputs):
    nc = _get_prog(2048, 8)
    in_maps = prep_in_maps(**inputs)
    res = run_bass_kernel_spmd(nc, in_maps, core_ids=list(range(NCORES)))
    return gather_out(res.results, B)


# revision 37
# speedup vs baseline: 1.4737x; 1.0017x over previous
"""LoRA multi-head attention kernel for 8 Trainium2 NeuronCores.

Problem: q = x_q@(Wq.T + Aq@Bq*2) + bq ; k = x_k@Wk.T + bk ;
         v = x_v@(Wv.T + Av@Bv*2) + bv ; MHA over 16 heads, D=64,
         out = attn_out @ Wo.T + bo.   Shapes: x [2048, 4, 1024].

Sharding: core c handles batch b = c//2 and head-group hg = c%2
(8 heads = 512 channels). LoRA weights are merged on the host
(mathematically exact), the 1/sqrt(D) score scale is folded into Wk/bk,
and x is transposed on the host so every matmul contracts over the
partition dimension. Each core computes a partial output
(its 512 channels through Wo); the host sums the two partials per batch.

Device pipeline per core (all matmul inputs bf16 except scores in fp8):
  - q/k projections write fp8e4m3 qT/kT [ch, tok] with a zeroed second
    "DoubleRow plane"; the QK^T matmuls then run in fp8 DoubleRow mode
    (plane 1 contributes exactly zero), at half PE cost.
  - exp runs on the Activation engine out of a 6-bank PSUM score buffer
    in 3-bank groups, writing bf16 ex tiles.
  - attn@v is oriented [s_tile=128, D+1] (ex stationary, [v|1] moving)
    so all 128 output partitions are used; column D is the softmax
    denominator; a single DVE divide normalizes, a PE transpose flips
    back to [ch, tok] for the bf16 output projection.
  - phase work is software-pipelined so the Activation engine (the
    serial-exp floor) runs continuously: q/k chunks 1-3 and the v
    projection are emitted as filler between attention units.
"""

import sys

import numpy as np

sys.path.insert(0, "/opt/trn_rl_repo")

from contextlib import ExitStack  # noqa: E402

import ml_dtypes  # noqa: E402

import concourse.bass as bass  # noqa: E402
import concourse.tile as tile  # noqa: E402
from concourse import bacc, mybir  # noqa: E402
from concourse.bass_utils import run_bass_kernel_spmd  # noqa: E402

F32 = mybir.dt.float32
BF16 = mybir.dt.bfloat16
F8 = mybir.dt.float8e4
AF = mybir.ActivationFunctionType
ALU = mybir.AluOpType
DR = mybir.MatmulPerfMode.DoubleRow

E = 1024
D = 64
NHC = 8            # heads per core
CH = NHC * D       # 512 output channels per core
KT = E // 128      # k-tiles over the E contraction
NCORES = 8
B = 4
BF = ml_dtypes.bfloat16


TUNE = {
    "warmup": 16,
    "b_early": 8.4,
    "b_mid": 4.4,
    "b_late": 4.4,
    "cost_v": 5.2,
    "cost_qk": 3.2,
    "cost_c": 3.0,
    "cbudget": 4.4,
    "early_units": 7,
    "mid_units": 24,
}


def build_program(S=2048, num_devices=8):
    TB = 256                        # token block for projections
    NTB = S // TB                   # 8
    NTT = S // 128                  # 16 t/tok tiles
    NSB = S // 512                  # 4 s-blocks
    NM = CH // 128                  # 4 ch chunks per core

    nc = bacc.Bacc(
        "TRN2", target_bir_lowering=False, debug=False, num_devices=num_devices
    )

    def dram(name, shape, out=False, dt=F32):
        kind = "ExternalOutput" if out else "ExternalInput"
        return nc.dram_tensor(name, shape, dt, kind=kind).ap()

    xq = dram("xq", [128, KT, S], dt=BF16)
    xk = dram("xk", [128, KT, S], dt=BF16)
    xv = dram("xv", [128, KT, S], dt=BF16)
    wq = dram("wq", [128, KT, CH], dt=BF16)
    wk = dram("wk", [128, KT, CH], dt=BF16)
    wv = dram("wv", [128, KT, CH], dt=BF16)
    wo = dram("wo", [128, NM, E // 512, 512], dt=BF16)
    bq = dram("bq", [128, NM])
    bk = dram("bk", [128, NM])
    bv = dram("bv", [128, CH], dt=BF16)
    bo = dram("bo", [128, E], dt=BF16)
    out = dram("out", [S, E], out=True, dt=BF16)

    with tile.TileContext(nc) as tc, ExitStack() as top:
        persist = top.enter_context(tc.tile_pool(name="persist", bufs=1))
        q8 = persist.tile([128, NM, S], F8)         # [ch%128, ch//128, tok]
        k8 = persist.tile([128, NM, S], F8)
        # innermost padded to 72 so the DoubleRow plane stride (8*72)
        # is a multiple of 64 (walrus ISA check on ldweights)
        vaug = persist.tile([128, NTT, NHC, 72], F8)   # [tok%128, tt, h, d|1]
        ones_sb = persist.tile([1, D], BF16)
        negone = persist.tile([128, 1], F32)
        ex0 = persist.tile([128, 16, 512], F8)
        aoTs = [persist.tile([128, NM, 512], BF16, name=f"aoT{s}") for s in range(NSB)]
        wv_sb = persist.tile([128, KT, CH], BF16)
        wo_sb = persist.tile([128, NM, E // 512, 512], BF16)
        bq_sb = persist.tile([128, NM], F32)
        bk_sb = persist.tile([128, NM], F32)
        bv_sb = persist.tile([128, CH], BF16)
        bo_sb = persist.tile([128, E], BF16)

        # DMA queue split: SP carries wk+xk (k path), the Activation
        # sequencer carries wq+xq then wv+xv+wo (q/v path); both x tensors
        # load as single full-tensor DMAs (few descriptors, early start).
        nc.sync.dma_start(out=bk_sb, in_=bk)
        nc.scalar.dma_start(out=bq_sb, in_=bq)
        warm = persist.tile([1, 256], BF16)
        nc.gpsimd.memset(warm, 1.0)
        nc.gpsimd.memset(vaug[:, :, :, D:D + 1], 1.0)
        nc.gpsimd.memset(ones_sb, 1.0)
        nc.gpsimd.memset(negone, -1.0)

        with tc.tile_pool(name="wqk", bufs=1) as wqk, \
             tc.tile_pool(name="xvs", bufs=4) as xvpool, \
             tc.tile_pool(name="pps", bufs=2, space="PSUM") as ppool:
            wq_sb = wqk.tile([128, KT, CH], BF16, tag="wq")
            wk_sb = wqk.tile([128, KT, CH], BF16, tag="wk")
            xq_sb = wqk.tile([128, KT, S], BF16, tag="xq")

            def _ld(dst, srct, nb):
                sl = slice(nb * TB, (nb + 1) * TB)
                nc.sync.dma_start(out=dst[:, :, sl], in_=srct[:, :, sl])

            def _xvld(nb):
                xt = xvpool.tile([128, KT, TB], BF16, tag="xv", name="xvt")
                nc.sync.dma_start(out=xt, in_=xv[:, :, nb * TB:(nb + 1) * TB])
                xv_tiles.append(xt)

            xv_tiles = []
            xk_src = {}

            def qk_task(which, hp, nb):
                """Project one [128ch, 256tok] chunk of q or k into fp8."""
                if which == "k":
                    w_sb, b_sb, dst = wk_sb, bk_sb, k8
                    if (hp, nb) in xk_src:
                        xs = xk_src.pop((hp, nb))
                        xsl = slice(nb * TB, (nb + 1) * TB)
                    else:
                        xs = xkpool.tile(
                            [128, KT, TB], BF16, tag="xk", name="xkt"
                        )
                        nc.sync.dma_start(
                            out=xs, in_=xk[:, :, nb * TB:(nb + 1) * TB]
                        )
                        xsl = slice(0, TB)
                else:
                    w_sb, b_sb, dst = wq_sb, bq_sb, q8
                    xs = xq_sb
                    xsl = slice(nb * TB, (nb + 1) * TB)
                pq = ppool.tile([128, 512], F32, tag="pp", name="pq")
                for k in range(KT):
                    nc.tensor.matmul(
                        pq[:, 0:TB],
                        (w_sb[:, k, hp * 128:(hp + 1) * 128]),
                        (xs[:, k, xsl]),
                        start=(k == 0),
                        stop=(k == KT - 1),
                    )
                nc.vector.tensor_scalar(
                    out=dst[:, hp, nb * TB:(nb + 1) * TB],
                    in0=pq[:, 0:TB],
                    scalar1=b_sb[:, hp:hp + 1],
                    scalar2=None,
                    op0=ALU.add,
                )

            def v_task(nb, mi):
                """Project one 128-token half-block of v into fp8 vaug."""
                xt = xv_tiles[nb]
                tt = nb * (TB // 128) + mi
                pv = ppool.tile([128, 512], F32, tag="pp", name="pv")
                for k in range(KT):
                    nc.tensor.matmul(
                        pv,
                        (xt[:, k, mi * 128:(mi + 1) * 128]),
                        (wv_sb[:, k, :]),
                        start=(k == 0),
                        stop=(k == KT - 1),
                    )
                nc.vector.tensor_add(
                    out=vaug[:, tt, :, 0:D],
                    in0=pv.rearrange("p (h d) -> p h d", d=D),
                    in1=bv_sb.rearrange("p (h d) -> p h d", d=D),
                )

            done_sb = {}        # sb -> finished (hp, h_in) units

            def av_task(hp, sb, h_in, ex):
                """attn@v (fp8 DoubleRow, two t-tiles per matmul) + softmax
                normalize for one head."""
                h = 2 * hp + h_in
                p0 = h_in * 64
                oa = ppool.tile([128, 512], F32, tag="pp", name="oa")
                for p in range(NTT // 2):
                    nc.tensor.matmul(
                        oa[0:D + 1, :],
                        (vaug[:, 2 * p:2 * p + 2, h, 0:D + 1]),
                        (ex[:, 2 * p:2 * p + 2, :]),
                        start=(p == 0),
                        stop=(p == NTT // 2 - 1),
                        perf_mode=DR,
                    )
                recip = npool.tile([1, 512], F32, tag="rc", name="recip")
                nc.vector.reciprocal(out=recip, in_=oa[D:D + 1, :])
                rb = npool.tile([D, 512], F32, tag="rbf", name="rb")
                nc.gpsimd.partition_broadcast(rb, recip)
                nc.vector.tensor_mul(
                    out=aoTs[sb][p0:p0 + 64, hp, :], in0=oa[0:D, :], in1=rb
                )
                done_sb.setdefault(sb, set()).add((hp, h_in))
                return sb if len(done_sb[sb]) == 2 * NM else None

            def c_task(sb, st):
                """Output projection for one 128-token tile."""
                stg = stgpool.tile([128, 2, 512], BF16, tag="stg", name="stg")
                mt = sb * 4 + st
                for nb2 in range(E // 512):
                    pc = ppool.tile([128, 512], F32, tag="pp", name="pc")
                    for kc in range(NM):
                        nc.tensor.matmul(
                            pc,
                            (aoTs[sb][:, kc, st * 128:(st + 1) * 128]),
                            (wo_sb[:, kc, nb2, :]),
                            start=(kc == 0),
                            stop=(kc == NM - 1),
                        )
                    nc.vector.tensor_add(
                        out=stg[:, nb2, :],
                        in0=pc,
                        in1=bo_sb[:, nb2 * 512:(nb2 + 1) * 512],
                    )
                    nc.gpsimd.dma_start(
                        out=out[mt * 128:(mt + 1) * 128,
                                nb2 * 512:(nb2 + 1) * 512],
                        in_=stg[:, nb2, :],
                    )

            # ---- startup: chunk 0 of q/k in a scoped pool whose 32KB is
            # reclaimed by the ex pool once the chunk-0 projections finish
            with tc.tile_pool(name="xk0", bufs=1) as xk0p:
                xk0_sb = xk0p.tile([128, KT, S], BF16, tag="xk0")
                # serial DMA order, sorted by consumer deadline
                nc.sync.dma_start(out=wk_sb[:, :, 0:128], in_=wk[:, :, 0:128])
                nc.sync.dma_start(out=wq_sb[:, :, 0:128], in_=wq[:, :, 0:128])
                for nb in range(2):
                    _ld(xk0_sb, xk, nb)
                for nb in range(2):
                    _ld(xq_sb, xq, nb)
                for nb in range(2, NTB):
                    _ld(xk0_sb, xk, nb)
                nc.sync.dma_start(out=wv_sb, in_=wv)
                _xvld(0)
                _xvld(1)
                _ld(xq_sb, xq, 2)
                _ld(xq_sb, xq, 3)
                _xvld(2)
                _xvld(3)
                _ld(xq_sb, xq, 4)
                _ld(xq_sb, xq, 5)
                nc.sync.dma_start(out=bv_sb, in_=bv)
                _xvld(4)
                _xvld(5)
                _ld(xq_sb, xq, 6)
                _ld(xq_sb, xq, 7)
                _xvld(6)
                _xvld(7)
                nc.sync.dma_start(
                    out=wk_sb[:, :, 128:CH], in_=wk[:, :, 128:CH]
                )
                nc.sync.dma_start(
                    out=wq_sb[:, :, 128:CH], in_=wq[:, :, 128:CH]
                )
                nc.sync.dma_start(out=bo_sb, in_=bo)

                # PE warmup: junk matmuls ramp the tensor engine clock while
                # the first x chunks stream in
                wps = ppool.tile([128, 512], F32, tag="pp", name="wps")
                for i in range(TUNE["warmup"]):
                    nc.tensor.matmul(
                        wps[0:64, 0:256], (warm[:, 0:D]), (warm),
                        start=True, stop=True,
                    )
                for nb in (0, 1):
                    xk_src[(0, nb)] = xk0_sb
                    qk_task("k", 0, nb)
                for nb in (0, 1):
                    qk_task("q", 0, nb)
                # unit 0 (hp0, sb0, h0) interleaved with the remaining
                # chunk-0 k-tasks: score group g only needs k tokens up to
                # tile 3g+2, so exp starts while k still streams in
                qk0_pending = list(range(2, NTB))
                for g0, glen in [(0, 3), (3, 3), (6, 3), (9, 3), (12, 3),
                                 (15, 1)]:
                    need_nb = min((g0 + glen + 2) // 2, NTB - 1)
                    while qk0_pending and qk0_pending[0] <= need_nb:
                        nb = qk0_pending.pop(0)
                        xk_src[(0, nb)] = xk0_sb
                        qk_task("k", 0, nb)
                    scp = ppool.tile([128, 3, 512], F32, tag="sc", name="scp")
                    for j in range(glen):
                        tt = g0 + j
                        nc.tensor.matmul(
                            scp[:, j, :],
                            (k8[0:64, 0, tt * 128:(tt + 1) * 128]
                             .unsqueeze(1).broadcast_to([64, 2, 128])),
                            (q8[0:64, 0, 0:512]
                             .unsqueeze(1).broadcast_to([64, 2, 512])),
                            start=True,
                            stop=True,
                            perf_mode=DR,
                        )
                    nc.scalar.activation(
                        out=ex0[:, g0:g0 + glen, :],
                        in_=scp[:, 0:glen, :],
                        func=AF.Exp,
                        bias=negone,
                    )
                for nb in qk0_pending:
                    xk_src[(0, nb)] = xk0_sb
                    qk_task("k", 0, nb)

            with tc.tile_pool(name="xks", bufs=3) as xkpool, \
                 tc.tile_pool(name="exs", bufs=5) as expool, \
                 tc.tile_pool(name="nrm", bufs=2) as npool, \
                 tc.tile_pool(name="stg", bufs=2) as stgpool:
                # fill queue: v interleaved with qk chunk 1, then chunks 2-3.
                # costs are approximate PE microseconds, used to budget how much
                # filler is emitted per attention unit so the Act engine (exp)
                # never starves.
                fill = [("v", (nb, mi), TUNE["cost_v"] / 2)
                        for nb in range(NTB) for mi in range(2)]
                for hp in range(1, NM):
                    fill += [("k", hp, nb, TUNE["cost_qk"]) for nb in range(NTB)]
                    fill += [("q", hp, nb, TUNE["cost_qk"]) for nb in range(NTB)]
                fill += [("q", 0, nb, TUNE["cost_qk"]) for nb in range(2, NTB)]
                v_left = 2 * NTB

                units = [
                    (hp, sb, h_in)
                    for hp in range(NM)
                    for sb in range(NSB)
                    for h_in in range(2)
                ]
                groups = [(0, 3), (3, 3), (6, 3), (9, 3), (12, 3), (15, 1)]
                av_queue = [(0, 0, 0, ex0)]
                c_queue = []

                def run_fill(t):
                    nonlocal v_left
                    if t[0] == "v":
                        v_task(*t[1])
                        v_left -= 1
                    else:
                        qk_task(t[0], t[1], t[2])

                budget_acc = [0.0]

                def emit_quanta(limit):
                    # attn@v keeps pace with exp (self-limited by ex bufs);
                    # small filler quanta keep PE fed without letting any
                    # single task starve the next score group
                    budget_acc[0] = min(budget_acc[0] + limit, 3.0 * limit)
                    while av_queue and v_left == 0:
                        t = av_queue.pop(0)
                        full = av_task(*t)
                        if full is not None:
                            c_queue.extend((full, st) for st in range(4))
                    while budget_acc[0] > 0:
                        if fill:
                            t = fill.pop(0)
                            run_fill(t)
                            budget_acc[0] -= t[-1]
                        elif c_queue:
                            c_task(*c_queue.pop(0))
                            budget_acc[0] -= TUNE["cost_c"]
                        else:
                            break

                for ui, (hp, sb, h_in) in enumerate(units):
                    if ui == 0:
                        continue  # unit 0 emitted inside the xk0 scope
                    if ui == 8:
                        nc.scalar.dma_start(out=wo_sb, in_=wo)
                    # force projection work this unit depends on: all k
                    # blocks of chunk hp, q blocks for this s-block only
                    for i in reversed([
                        i for i, t in enumerate(fill)
                        if (t[0] == "k" and t[1] == hp)
                        or (t[0] == "q" and t[1] == hp
                            and t[2] in (2 * sb, 2 * sb + 1))
                    ]):
                        run_fill(fill.pop(i))
                    per_group = (TUNE["b_early"] if ui < TUNE["early_units"]
                                 else TUNE["b_mid"] if ui < TUNE["mid_units"]
                                 else TUNE["b_late"]) / 6.0
                    if ui >= 28:
                        per_group = 2.0
                    p0 = h_in * 64
                    ex = expool.tile([128, NTT, 512], F8, tag="ex", name="ex")
                    for g0, glen in groups:
                        scp = ppool.tile([128, 3, 512], F32, tag="sc", name="scp")
                        for j in range(glen):
                            tt = g0 + j
                            nc.tensor.matmul(
                                scp[:, j, :],
                                (k8[p0:p0 + 64, hp, tt * 128:(tt + 1) * 128]
                                 .unsqueeze(1).broadcast_to([64, 2, 128])),
                                (q8[p0:p0 + 64, hp, sb * 512:(sb + 1) * 512]
                                 .unsqueeze(1).broadcast_to([64, 2, 512])),
                                start=True,
                                stop=True,
                                perf_mode=DR,
                            )
                        nc.scalar.activation(
                            out=ex[:, g0:g0 + glen, :],
                            in_=scp[:, 0:glen, :],
                            func=AF.Exp,
                            bias=negone,
                        )
                        emit_quanta(per_group)
                    av_queue.append((hp, sb, h_in, ex))

                while fill:
                    run_fill(fill.pop(0))
                while av_queue:
                    t = av_queue.pop(0)
                    full = av_task(*t)
                    if full is not None:
                        c_queue += [(full, st) for st in range(4)]
                while c_queue:
                    c_task(*c_queue.pop(0))

    nc.compile()
    return nc


_PROG = {}


def _get_prog(S=2048, num_devices=8):
    key = (S, num_devices)
    if key not in _PROG:
        _PROG[key] = build_program(S, num_devices)
    return _PROG[key]


def _tile_x(x2d):
    # [S, E] slice -> [128, KT, S] with element (p, k, t) = x2d[t, k*128+p]
    S = x2d.shape[0]
    xt = np.ascontiguousarray(x2d.T)
    return np.ascontiguousarray(
        xt.reshape(KT, 128, S).transpose(1, 0, 2).astype(BF)
    )


def _tile_w(weff, ch0):
    w = weff[:, ch0:ch0 + CH]
    return np.ascontiguousarray(
        w.reshape(KT, 128, CH).transpose(1, 0, 2).astype(BF)
    )


def prep_in_maps(x_q, x_k, x_v, Wq, bq, Aq, Bq, Wk, bk, Wv, bv, Av, Bv, Wo, bo):
    x_q = np.asarray(x_q, np.float32)
    x_k = np.asarray(x_k, np.float32)
    x_v = np.asarray(x_v, np.float32)
    scaling = 2.0  # lora_alpha / r = 32 / 16
    wq_eff = (
        (np.asarray(Wq).T + (np.asarray(Aq) @ np.asarray(Bq)) * scaling) * 0.25
    ).astype(np.float32)
    wv_eff = (np.asarray(Wv).T + (np.asarray(Av) @ np.asarray(Bv)) * scaling).astype(
        np.float32
    )
    wk_s = (np.asarray(Wk).T / 4.0).astype(np.float32)
    bk_s = (np.asarray(bk) / 4.0).astype(np.float32)
    bq = np.asarray(bq, np.float32) * 0.25
    bv = np.asarray(bv, np.float32)
    bo = np.asarray(bo, np.float32)
    woT = np.ascontiguousarray(np.asarray(Wo).T.astype(np.float32))
    identity = np.eye(128, dtype=BF)

    nbatch = x_q.shape[1]
    in_maps = []
    for c in range(2 * nbatch):
        b = c // 2
        hg = c % 2
        ch0 = hg * CH
        wo_c = np.ascontiguousarray(
                woT[ch0:ch0 + CH, :].reshape(CH // 128, 128, E // 512, 512)
                .transpose(1, 0, 2, 3).astype(BF)
        )
        in_maps.append({
                "xq": _tile_x(x_q[:, b, :]),
                "xk": _tile_x(x_k[:, b, :]),
                "xv": _tile_x(x_v[:, b, :]),
                "wq": _tile_w(wq_eff, ch0),
                "wk": _tile_w(wk_s, ch0),
                "wv": _tile_w(wv_eff, ch0),
                "wo": wo_c,
                "bq": np.ascontiguousarray(bq[ch0:ch0 + CH].reshape(CH // 128, 128).T),
                "bk": np.ascontiguousarray(bk_s[ch0:ch0 + CH].reshape(CH // 128, 128).T),
                "bv": np.ascontiguousarray(np.broadcast_to(bv[ch0:ch0 + CH], (128, CH)).astype(BF)),
                "ident": identity,
                "bo": (
                    np.ascontiguousarray(np.broadcast_to(bo, (128, E)).astype(BF))
                    if hg == 0
                    else np.zeros((128, E), BF)
                ),
        })
    return in_maps


def gather_out(results, nbatch):
    return np.stack(
        [
                results[2 * b]["out"].astype(np.float32)
                + results[2 * b + 1]["out"].astype(np.float32)
                for b in range(nbatch)
        ],
        axis=1,
    )


def kernel(**inputs):
    nc = _get_prog(2048, 8)
    in_maps = prep_in_maps(**inputs)
    res = run_bass_kernel_spmd(nc, in_maps, core_ids=list(range(NCORES)))
    return gather_out(res.results, B)
